# revision 14
# baseline (speedup 1.0000x reference)
"""Trainium2 Bass kernel for nn_DCConv3dKernelPolynomials.

Computes out[m,n,b,p] = sum_k coeff[m,n,k] * psi_k(position[b,p,:])
where psi_k are the 23 real hydrogen-like wavefunctions (n<=4, l<=2).

Key math: with u=x/r, v=y/r, w=z/r, the angular factors are pure
polynomials, so the device only needs sqrt/reciprocal/exp plus
polynomial arithmetic.  All four radial exponentials come from one
exp:  t=e^{-r/12}; e4=t^3, e3=t^4, e2=t^6, e1=t^12.

Perf design (HBM-write-roofline bound, ~94us/core):
  * fp16 output (rel-err ~5e-4, gate is 2e-2) halves the write wall
    vs f32: 32 MiB/core at ~358 GB/s.
  * fp16 K=23 matmul, packed 4-up into 32-row PE strips via
    tile_position=(32i,0): 4 point-blocks computed concurrently, so
    the PE is never the bottleneck (no HAM-throttle sensitivity).
  * poly transposes packed 4-up into 32-col PE strips, written
    straight into a borrowed 4-bank PSUM tile laid out as poly_t4.
  * PSUM->SBUF fp32->fp16 copies on 4-bank [128,2048] tiles (amortizes
    the per-op bubble), split vector/scalar 26/38 to balance engines.
  * per-mt [128,4096] fp16 stages -> 32 x 1 MiB HWDGE DMAs on the
    sync ring (8 KiB/partition contiguous lines).

Sharding: batch b -> core b (8 cores, 4096 points each).
Point order on core: point q lives at (partition p=q%128, chunk
c=q//128), so transposed chunk c lands at poly_t4 block i=c//8
(partitions 32i..32i+22), cols 128*(c%8)..+128 == natural column q.
"""

import math

import numpy as np

B = 8
PTS = 4096            # points per core
OUTC = INC = 64
MN = OUTC * INC       # 4096
NB = 23               # basis functions
NCORES = 8
NCH = 32              # chunks of 128 points
NMT = MN // 128       # 32 mn tiles
NBLK = 4              # point blocks (PE row strips)
PPB = PTS // NBLK     # 1024 points per block
SCALAR_COPIES = 37    # of the 64 stage copies, how many go to ACT


def _combos():
    combos = []
    for n in range(1, 5):
        for k in range(3):
            for m in range(-3, 4):
                if abs(m) <= k and k < n:
                    combos.append((n, k, m))
    return combos


COMBOS = _combos()
assert len(COMBOS) == NB


def _basis_scales():
    """Per-basis constant c_k so that psi_k = c_k * Rb_dev(n,l) * A_dev(l,m).

    Rb_dev / A_dev are the *unnormalized* tile products computed on device:
      Rb_dev(1,0)=e1, Rb_dev(2,0)=(2-r)e2, Rb_dev(2,1)=r*e2,
      Rb_dev(3,0)=(2r-(2/9)r^2-3)e3   [= -L_2^1(2r/3)e3]
      Rb_dev(3,1)=((8/3)r-(4/9)r^2)e3, Rb_dev(3,2)=r^2*e3,
      Rb_dev(4,0)=(4-3r+r^2/2-r^3/48)e4,
      Rb_dev(4,1)=(5r-(5/4)r^2+r^3/16)e4, Rb_dev(4,2)=(6-r/2)r^2*e4
      A_dev: 1, w, u, v, 3w^2-1, wu, wv, u^2-v^2, uv
    """
    fourpi = 4.0 * math.pi
    K00 = math.sqrt(1.0 / fourpi)
    K10 = math.sqrt(3.0 / fourpi)
    K11 = math.sqrt(3.0 / (2.0 * fourpi))
    K20 = math.sqrt(5.0 / fourpi)
    K21 = math.sqrt(5.0 / (6.0 * fourpi))
    K22 = math.sqrt(5.0 / (24.0 * fourpi))
    s2 = math.sqrt(2.0)

    def norm_r(n, l):
        return math.sqrt(
            (2.0 / n) ** 3 * math.factorial(n - l - 1)
            / (2 * n * math.factorial(n + l))
        )

    f = {
        (1, 0): norm_r(1, 0),
        (2, 0): norm_r(2, 0),
        (2, 1): norm_r(2, 1),
        (3, 0): -norm_r(3, 0),
        (3, 1): norm_r(3, 1),
        (3, 2): norm_r(3, 2) * (4.0 / 9.0),
        (4, 0): norm_r(4, 0),
        (4, 1): norm_r(4, 1),
        (4, 2): norm_r(4, 2) * 0.25,
    }
    a = {
        (0, 0): K00,
        (1, 0): K10,
        (1, 1): -s2 * K11,
        (1, -1): -s2 * K11,
        (2, 0): 0.5 * K20,
        (2, 1): -3.0 * s2 * K21,
        (2, -1): -3.0 * s2 * K21,
        (2, 2): 3.0 * s2 * K22,
        (2, -2): 6.0 * s2 * K22,
    }
    return np.array(
        [f[(n, l)] * a[(l, m)] for (n, l, m) in COMBOS], dtype=np.float64
    )


def poly_host(position):
    """Numpy replica of the device basis recipe (for self-checking)."""
    pos = np.asarray(position, dtype=np.float32)
    x, y, z = pos[..., 0], pos[..., 1], pos[..., 2]
    r2 = x * x + y * y + z * z
    r = np.sqrt(r2)
    ir = 1.0 / r
    u, v, w = x * ir, y * ir, z * ir
    e1, e2, e3, e4 = np.exp(-r), np.exp(-r / 2), np.exp(-r / 3), np.exp(-r / 4)
    rr = r * r
    A = {
        (0, 0): np.ones_like(r),
        (1, 0): w, (1, 1): u, (1, -1): v,
        (2, 0): 3 * w * w - 1, (2, 1): w * u, (2, -1): w * v,
        (2, 2): u * u - v * v, (2, -2): u * v,
    }
    Rb = {
        (1, 0): e1,
        (2, 0): (2 - r) * e2,
        (2, 1): r * e2,
        (3, 0): (2 * r - (2.0 / 9.0) * rr - 3) * e3,
        (3, 1): ((8.0 / 3.0) * r - (4.0 / 9.0) * rr) * e3,
        (3, 2): rr * e3,
        (4, 0): (4 - 3 * r + rr / 2 - rr * r / 48) * e4,
        (4, 1): (5 * r - 1.25 * rr + rr * r / 16) * e4,
        (4, 2): (6 - r / 2) * rr * e4,
    }
    c = _basis_scales()
    return np.stack(
        [
            (c[k] * Rb[(n, l)] * A[(l, m)]).astype(np.float32)
            for k, (n, l, m) in enumerate(COMBOS)
        ],
        axis=-1,
    )


_PROGRAM = None


def _build_program():
    import concourse.bacc as bacc
    import concourse.tile as tile
    from concourse import mybir
    from concourse.bass import ts
    from concourse.masks import make_identity

    f32 = mybir.dt.float32
    f16 = mybir.dt.float16
    AF = mybir.ActivationFunctionType
    ALU = mybir.AluOpType

    nc = bacc.Bacc(trn_type="TRN2")
    pos_d = nc.dram_tensor("position", [128, 96], f32, kind="ExternalInput")
    coefft_d = nc.dram_tensor("coefft4", [128, MN], f16, kind="ExternalInput")
    out_d = nc.dram_tensor("out", [MN, PTS], f16, kind="ExternalOutput")

    with tile.TileContext(nc) as tc:
        with (
            tc.tile_pool(name="const", bufs=1) as const,
            tc.tile_pool(name="pw", bufs=1) as pw,
            tc.tile_pool(name="stage", bufs=4) as stage_pool,
            tc.tile_pool(name="psum", bufs=2, space="PSUM") as psum,
        ):
            # inputs on the HWDGE sync ring (gpsimd is busy with memsets)
            xyz = const.tile([128, 96], f32, tag="xyz", name="xyz")
            nc.sync.dma_start(out=xyz[:], in_=pos_d[:, :])
            coefft4 = const.tile([128, MN], f16, tag="coefft4", name="coefft4_sb")
            nc.sync.dma_start(out=coefft4[:], in_=coefft_d[:, :])

            ident = const.tile([128, 128], f16, tag="ident", name="ident")
            make_identity(nc, ident[:])

            xyz3 = xyz[:].rearrange("p (c t) -> p c t", t=3)

            def T(tag):
                return pw.tile([128, NCH], f32, tag=tag, name=tag)[:]

            def bcastn(ap2d, n):
                import concourse.bass as bass
                return bass.AP(
                    tensor=ap2d.tensor,
                    offset=ap2d.offset,
                    ap=[ap2d.ap[0], [0, n], ap2d.ap[1]],
                )

            # basis values land in poly16[:, c, k] fp16; k padded to 32 and
            # replicated x4 so each transposed chunk fills all 128 PSUM
            # partitions with psi_k at rows 32i+k (base partition 0).
            poly16p = const.tile([128, NCH, 32], f16, tag="poly16", name="poly16")
            nc.gpsimd.memset(poly16p[:, :, NB:32], 0.0)
            poly_rep = const.tile(
                [128, NCH, NBLK, 32], f16, tag="poly_rep", name="poly_rep"
            )
            poly_t4 = const.tile([128, PTS], f16, tag="poly_t4", name="poly_t4")

            (r2, r, ir, rr, lnr2, t1, t2, e2, e3, e4, uu, vv, t20, rb21,
             p30, rb31, rb32, p40, p40b, rb41, rb42) = (
                T(t) for t in (
                    "r2 r ir rr lnr2 t1 t2 e2 e3 e4 uu vv t20 rb21 "
                    "p30 rb31 rb32 p40 p40b rb41 rb42"
                ).split()
            )
            vwu_t = pw.tile([128, 3, NCH], f32, tag="vwu", name="vwu")[:]
            ang5_t = pw.tile([128, 5, NCH], f32, tag="ang5", name="ang5")[:]

            def pointwise_half(h):
                """Compute psi into poly16[:, 16h:16h+16, :]."""
                sl = slice(16 * h, 16 * (h + 1))

                def H(t2d):
                    return t2d[:, sl]

                x, y, z = (xyz3[:, sl, t] for t in range(3))
                hr2, hr, hir, hrr, hlnr2 = (
                    H(t) for t in (r2, r, ir, rr, lnr2)
                )
                nc.vector.tensor_mul(hr2, x, x)
                tAh, tBh = H(uu), H(vv)   # reuse as scratch pre-uu/vv
                nc.vector.tensor_mul(tAh, y, y)
                nc.vector.tensor_add(hr2, hr2, tAh)
                nc.vector.tensor_mul(tBh, z, z)
                nc.vector.tensor_add(hr2, hr2, tBh)
                nc.scalar.activation(hlnr2, hr2, AF.Ln)
                nc.scalar.activation(hr, hlnr2, AF.Exp, scale=0.5)
                nc.scalar.activation(hir, hlnr2, AF.Exp, scale=-0.5)

                # radial exponentials from one exp: t1=e^{-r/12}
                ht1, ht2, he2, he3, he4 = (H(t) for t in (t1, t2, e2, e3, e4))
                nc.scalar.activation(ht1, hr, AF.Exp, scale=-1.0 / 12.0)
                nc.vector.tensor_mul(ht2, ht1, ht1)
                nc.vector.tensor_mul(he4, ht2, ht1)
                nc.vector.tensor_mul(he3, ht2, ht2)
                nc.vector.tensor_mul(he2, he3, ht2)

                vwu = vwu_t[:, :, sl]
                ang5 = ang5_t[:, :, sl]
                v, w, u = vwu[:, 0, :], vwu[:, 1, :], vwu[:, 2, :]
                uv, wv, a20, wu, a22 = (ang5[:, i, :] for i in range(5))
                nc.vector.tensor_mul(v, y, hir)
                nc.vector.tensor_mul(w, z, hir)
                nc.vector.tensor_mul(u, x, hir)
                nc.vector.tensor_mul(hrr, hr, hr)

                huu, hvv = H(uu), H(vv)
                nc.vector.tensor_mul(a20, w, w)
                nc.vector.tensor_scalar(a20, a20, 3.0, -1.0, ALU.mult, ALU.add)
                nc.vector.tensor_mul(huu, u, u)
                nc.vector.tensor_mul(hvv, v, v)
                nc.vector.tensor_sub(a22, huu, hvv)
                nc.vector.tensor_mul(uv, u, v)
                nc.vector.tensor_mul(wu, w, u)
                nc.vector.tensor_mul(wv, w, v)

                poly_s = poly16p[:, sl, 0:NB]
                slot = [poly16p[:, sl, k] for k in range(NB)]
                poly_kc = poly_s.rearrange("p c k -> p k c")

                nc.vector.tensor_mul(slot[0], he2, he2)          # e1
                ht20 = H(t20)
                nc.vector.tensor_scalar(ht20, hr, -1.0, 2.0, ALU.mult, ALU.add)
                nc.vector.tensor_mul(slot[1], ht20, he2)
                hrb21 = H(rb21)
                nc.vector.tensor_mul(hrb21, hr, he2)
                nc.vector.tensor_mul(poly_kc[:, 2:5, :], bcastn(hrb21, 3), vwu)
                hp30 = H(p30)
                nc.vector.tensor_scalar(hp30, hrr, 2.0 / 9.0, 3.0, ALU.mult, ALU.add)
                nc.vector.scalar_tensor_tensor(
                    hp30, hr, 2.0, hp30, ALU.mult, ALU.subtract
                )
                nc.vector.tensor_mul(slot[5], hp30, he3)
                hrb31 = H(rb31)
                nc.vector.tensor_scalar(
                    hrb31, hr, -4.0 / 9.0, 8.0 / 3.0, ALU.mult, ALU.add
                )
                nc.vector.tensor_mul(hrb31, hrb31, hr)
                nc.vector.tensor_mul(hrb31, hrb31, he3)
                nc.vector.tensor_mul(poly_kc[:, 6:9, :], bcastn(hrb31, 3), vwu)
                hrb32 = H(rb32)
                nc.vector.tensor_mul(hrb32, hrr, he3)
                nc.vector.tensor_mul(poly_kc[:, 9:14, :], bcastn(hrb32, 5), ang5)
                hp40, hp40b = H(p40), H(p40b)
                nc.vector.tensor_scalar(hp40, hr, -1.0 / 48.0, 0.5, ALU.mult, ALU.add)
                nc.vector.tensor_mul(hp40, hp40, hrr)
                nc.vector.tensor_scalar(hp40b, hr, -3.0, 4.0, ALU.mult, ALU.add)
                nc.vector.tensor_add(hp40, hp40, hp40b)
                nc.vector.tensor_mul(slot[14], hp40, he4)
                hrb41 = H(rb41)
                nc.vector.tensor_scalar(hrb41, hr, 1.0 / 16.0, -1.25, ALU.mult, ALU.add)
                nc.vector.tensor_mul(hrb41, hrb41, hr)
                nc.vector.tensor_scalar(hrb41, hrb41, 5.0, None, ALU.add)
                nc.vector.tensor_mul(hrb41, hrb41, hr)
                nc.vector.tensor_mul(hrb41, hrb41, he4)
                nc.vector.tensor_mul(poly_kc[:, 15:18, :], bcastn(hrb41, 3), vwu)
                hrb42 = H(rb42)
                nc.vector.tensor_scalar(hrb42, hr, -0.5, 6.0, ALU.mult, ALU.add)
                nc.vector.tensor_mul(hrb42, hrb42, hrr)
                nc.vector.tensor_mul(hrb42, hrb42, he4)
                nc.vector.tensor_mul(poly_kc[:, 18:23, :], bcastn(hrb42, 5), ang5)

            # ---- emission helpers ----------------------------------------
            def rep_half(h):
                sl = slice(16 * h, 16 * (h + 1))
                for rep in range(NBLK):
                    eng = nc.vector.tensor_copy if rep % 2 == 0 else nc.scalar.copy
                    eng(poly_rep[:, sl, rep, :], poly16p[:, sl, :])

            def transpose_half(h, drain_eng):
                tr = psum.tile([128, 2048], f32, tag="mm", name="tr_ps")
                trv = tr[:].bitcast(f16)
                for cc in range(16):
                    c = 16 * h + cc
                    nc.tensor.transpose(
                        trv[:, 128 * cc:128 * (cc + 1)],
                        poly_rep[:, c, :, :],
                        ident[:],
                    )
                drain_eng(poly_t4[:, ts(h, 2048)], trv[:, 0:2048])

            # per (mt, h): 4 strip-concurrent MMs; strip i computes point
            # window w=4h+i (psum bank i == stage cols 2048h+512i == points
            # 2048h+512i.. -- identity order), one copy, one 512 KiB DMA.
            def mm_group(mt, h, use_scalar):
                ps = psum.tile([128, 2048], f32, tag="mm", name="mmps")
                for i in range(NBLK):
                    w = 4 * h + i
                    nc.tensor.matmul(
                        ps[:, ts(i, 512)],
                        lhsT=coefft4[32 * i:32 * i + NB, ts(mt, 128)],
                        rhs=poly_t4[32 * i:32 * i + NB, ts(w, 512)],
                        start=True,
                        stop=True,
                        tile_position=(32 * i, 0),
                    )
                stage = stage_pool.tile([128, 2048], f16, tag="stage", name="stage")
                if use_scalar:
                    nc.scalar.copy(stage[:], ps[:])
                else:
                    nc.vector.tensor_copy(stage[:], ps[:])
                nc.sync.dma_start(
                    out=out_d[ts(mt, 128), ts(h, 2048)], in_=stage[:]
                )

            # ---- emission order: h0 chain; 4 early h0 groups run on the
            # PE between the two transpose batches; h1 groups lag h0 by 4.
            pointwise_half(0)
            rep_half(0)
            transpose_half(0, nc.scalar.copy)
            pointwise_half(1)
            rep_half(1)
            for mt in range(4):
                mm_group(mt, 0, use_scalar=(mt % 2 == 0))
            transpose_half(1, nc.vector.tensor_copy)

            order = []
            for m in range(4, NMT + 4):
                if m < NMT:
                    order.append((m, 0))
                order.append((m - 4, 1))
            scal_cum = 0
            n_rest = len(order)
            for idx, (mt, h) in enumerate(order):
                s_next = ((idx + 1) * (SCALAR_COPIES - 2)) // n_rest
                use_scalar = s_next > scal_cum
                scal_cum = s_next
                mm_group(mt, h, use_scalar)

    nc.finalize()
    return nc


def _get_program():
    global _PROGRAM
    if _PROGRAM is None:
        _PROGRAM = _build_program()
    return _PROGRAM


def _prep_inputs(position, coefficients):
    pos = np.asarray(position, dtype=np.float32)
    coeff = np.asarray(coefficients, dtype=np.float32)
    assert pos.shape == (B, PTS, 3) and coeff.shape == (OUTC, INC, NB)
    c = _basis_scales().astype(np.float32)
    C = (coeff * c).reshape(MN, NB).T.astype(np.float16)  # [23, 4096]
    coefft4 = np.zeros((128, MN), dtype=np.float16)
    for i in range(NBLK):
        coefft4[32 * i:32 * i + NB] = C
    # point q -> (partition q%128, chunk q//128)
    return [
        {
            "position": np.ascontiguousarray(
                pos[b].reshape(NCH, 128, 3).transpose(1, 0, 2).reshape(128, 96)
            ),
            "coefft4": coefft4,
        }
        for b in range(B)
    ]


def _assemble(results):
    return np.stack(
        [
            np.asarray(r["out"]).astype(np.float32).reshape(OUTC, INC, PTS)
            for r in results
        ],
        axis=2,
    )


def kernel(position, coefficients):
    from concourse import bass_utils

    nc = _get_program()
    in_maps = _prep_inputs(position, coefficients)
    res = bass_utils.run_bass_kernel_spmd(nc, in_maps, core_ids=list(range(NCORES)))
    return _assemble(res.results)


def kernel_traced(position, coefficients, trace_cores=None):
    """Like kernel() but captures an NTFF trace; returns (out, results)."""
    from concourse import bass_utils

    nc = _get_program()
    in_maps = _prep_inputs(position, coefficients)
    res = bass_utils.run_bass_kernel_spmd(
        nc,
        in_maps,
        core_ids=list(range(NCORES)),
        trace=True,
        trace_cores=trace_cores,
    )
    return _assemble(res.results), res


# revision 15
# speedup vs baseline: 1.0922x; 1.0922x over previous
"""Trainium2 Bass kernel for nn_DCConv3dKernelPolynomials.

Computes out[m,n,b,p] = sum_k coeff[m,n,k] * psi_k(position[b,p,:])
where psi_k are the 23 real hydrogen-like wavefunctions (n<=4, l<=2).

Key math: with u=x/r, v=y/r, w=z/r, the angular factors are pure
polynomials, so the device only needs sqrt/reciprocal/exp plus
polynomial arithmetic.  All four radial exponentials come from one
exp:  t=e^{-r/12}; e4=t^3, e3=t^4, e2=t^6, e1=t^12.

Perf design (HBM-write-roofline bound, ~94us/core):
  * fp16 output (rel-err ~5e-4, gate is 2e-2) halves the write wall
    vs f32: 32 MiB/core at ~358 GB/s.
  * fp16 K=23 matmul, packed 4-up into 32-row PE strips via
    tile_position=(32i,0): 4 point-blocks computed concurrently, so
    the PE is never the bottleneck (no HAM-throttle sensitivity).
  * poly transposes packed 4-up into 32-col PE strips, written
    straight into a borrowed 4-bank PSUM tile laid out as poly_t4.
  * PSUM->SBUF fp32->fp16 copies on 4-bank [128,2048] tiles (amortizes
    the per-op bubble), split vector/scalar 26/38 to balance engines.
  * per-mt [128,4096] fp16 stages -> 32 x 1 MiB HWDGE DMAs on the
    sync ring (8 KiB/partition contiguous lines).

Sharding: batch b -> core b (8 cores, 4096 points each).
Point order on core: point q lives at (partition p=q%128, chunk
c=q//128), so transposed chunk c lands at poly_t4 block i=c//8
(partitions 32i..32i+22), cols 128*(c%8)..+128 == natural column q.
"""

import math

import numpy as np

B = 8
PTS = 4096            # points per core
OUTC = INC = 64
MN = OUTC * INC       # 4096
NB = 23               # basis functions
NCORES = 8
NCH = 32              # chunks of 128 points
NMT = MN // 128       # 32 mn tiles
NBLK = 4              # point blocks (PE row strips)
PPB = PTS // NBLK     # 1024 points per block
SCALAR_COPIES = 37    # of the 64 stage copies, how many go to ACT


def _combos():
    combos = []
    for n in range(1, 5):
        for k in range(3):
            for m in range(-3, 4):
                if abs(m) <= k and k < n:
                    combos.append((n, k, m))
    return combos


COMBOS = _combos()
assert len(COMBOS) == NB


def _basis_scales():
    """Per-basis constant c_k so that psi_k = c_k * Rb_dev(n,l) * A_dev(l,m).

    Rb_dev / A_dev are the *unnormalized* tile products computed on device:
      Rb_dev(1,0)=e1, Rb_dev(2,0)=(2-r)e2, Rb_dev(2,1)=r*e2,
      Rb_dev(3,0)=(2r-(2/9)r^2-3)e3   [= -L_2^1(2r/3)e3]
      Rb_dev(3,1)=((8/3)r-(4/9)r^2)e3, Rb_dev(3,2)=r^2*e3,
      Rb_dev(4,0)=(4-3r+r^2/2-r^3/48)e4,
      Rb_dev(4,1)=(5r-(5/4)r^2+r^3/16)e4, Rb_dev(4,2)=(6-r/2)r^2*e4
      A_dev: 1, w, u, v, 3w^2-1, wu, wv, u^2-v^2, uv
    """
    fourpi = 4.0 * math.pi
    K00 = math.sqrt(1.0 / fourpi)
    K10 = math.sqrt(3.0 / fourpi)
    K11 = math.sqrt(3.0 / (2.0 * fourpi))
    K20 = math.sqrt(5.0 / fourpi)
    K21 = math.sqrt(5.0 / (6.0 * fourpi))
    K22 = math.sqrt(5.0 / (24.0 * fourpi))
    s2 = math.sqrt(2.0)

    def norm_r(n, l):
        return math.sqrt(
            (2.0 / n) ** 3 * math.factorial(n - l - 1)
            / (2 * n * math.factorial(n + l))
        )

    f = {
        (1, 0): norm_r(1, 0),
        (2, 0): norm_r(2, 0),
        (2, 1): norm_r(2, 1),
        (3, 0): -norm_r(3, 0),
        (3, 1): norm_r(3, 1),
        (3, 2): norm_r(3, 2) * (4.0 / 9.0),
        (4, 0): norm_r(4, 0),
        (4, 1): norm_r(4, 1),
        (4, 2): norm_r(4, 2) * 0.25,
    }
    a = {
        (0, 0): K00,
        (1, 0): K10,
        (1, 1): -s2 * K11,
        (1, -1): -s2 * K11,
        (2, 0): 0.5 * K20,
        (2, 1): -3.0 * s2 * K21,
        (2, -1): -3.0 * s2 * K21,
        (2, 2): 3.0 * s2 * K22,
        (2, -2): 6.0 * s2 * K22,
    }
    return np.array(
        [f[(n, l)] * a[(l, m)] for (n, l, m) in COMBOS], dtype=np.float64
    )


def poly_host(position):
    """Numpy replica of the device basis recipe (for self-checking)."""
    pos = np.asarray(position, dtype=np.float32)
    x, y, z = pos[..., 0], pos[..., 1], pos[..., 2]
    r2 = x * x + y * y + z * z
    r = np.sqrt(r2)
    ir = 1.0 / r
    u, v, w = x * ir, y * ir, z * ir
    e1, e2, e3, e4 = np.exp(-r), np.exp(-r / 2), np.exp(-r / 3), np.exp(-r / 4)
    rr = r * r
    A = {
        (0, 0): np.ones_like(r),
        (1, 0): w, (1, 1): u, (1, -1): v,
        (2, 0): 3 * w * w - 1, (2, 1): w * u, (2, -1): w * v,
        (2, 2): u * u - v * v, (2, -2): u * v,
    }
    Rb = {
        (1, 0): e1,
        (2, 0): (2 - r) * e2,
        (2, 1): r * e2,
        (3, 0): (2 * r - (2.0 / 9.0) * rr - 3) * e3,
        (3, 1): ((8.0 / 3.0) * r - (4.0 / 9.0) * rr) * e3,
        (3, 2): rr * e3,
        (4, 0): (4 - 3 * r + rr / 2 - rr * r / 48) * e4,
        (4, 1): (5 * r - 1.25 * rr + rr * r / 16) * e4,
        (4, 2): (6 - r / 2) * rr * e4,
    }
    c = _basis_scales()
    return np.stack(
        [
            (c[k] * Rb[(n, l)] * A[(l, m)]).astype(np.float32)
            for k, (n, l, m) in enumerate(COMBOS)
        ],
        axis=-1,
    )


_PROGRAM = None


def _build_program():
    import concourse.bacc as bacc
    import concourse.tile as tile
    from concourse import mybir
    from concourse.bass import ts
    from concourse.masks import make_identity

    f32 = mybir.dt.float32
    f16 = mybir.dt.float16
    AF = mybir.ActivationFunctionType
    ALU = mybir.AluOpType

    nc = bacc.Bacc(trn_type="TRN2")
    pos_d = nc.dram_tensor("position", [128, 96], f32, kind="ExternalInput")
    coefft_d = nc.dram_tensor("coefft4", [128, MN], f16, kind="ExternalInput")
    out_d = nc.dram_tensor("out", [MN, PTS], f16, kind="ExternalOutput")

    with tile.TileContext(nc) as tc:
        with (
            tc.tile_pool(name="const", bufs=1) as const,
            tc.tile_pool(name="pw", bufs=1) as pw,
            tc.tile_pool(name="stage", bufs=4) as stage_pool,
            tc.tile_pool(name="psum", bufs=2, space="PSUM") as psum,
        ):
            # inputs on the HWDGE sync ring (gpsimd is busy with memsets)
            xyz = const.tile([128, 96], f32, tag="xyz", name="xyz")
            nc.sync.dma_start(out=xyz[:], in_=pos_d[:, :])
            coefft4 = const.tile([128, MN], f16, tag="coefft4", name="coefft4_sb")
            nc.sync.dma_start(out=coefft4[:], in_=coefft_d[:, :])

            ident = const.tile([128, 128], f16, tag="ident", name="ident")
            make_identity(nc, ident[:])

            xyz3 = xyz[:].rearrange("p (c t) -> p c t", t=3)

            def T(tag):
                return pw.tile([128, NCH], f32, tag=tag, name=tag)[:]

            def bcastn(ap2d, n):
                import concourse.bass as bass
                return bass.AP(
                    tensor=ap2d.tensor,
                    offset=ap2d.offset,
                    ap=[ap2d.ap[0], [0, n], ap2d.ap[1]],
                )

            # basis values land in poly16[:, c, k] fp16; k padded to 32 and
            # replicated x4 so each transposed chunk fills all 128 PSUM
            # partitions with psi_k at rows 32i+k (base partition 0).
            poly16p = const.tile([128, NCH, 32], f16, tag="poly16", name="poly16")
            nc.gpsimd.memset(poly16p[:, :, NB:32], 0.0)
            poly_rep = const.tile(
                [128, NCH, NBLK, 32], f16, tag="poly_rep", name="poly_rep"
            )
            poly_t4 = const.tile([128, PTS], f16, tag="poly_t4", name="poly_t4")

            (r2, r, ir, rr, lnr2, t1, t2, e2, e3, e4, uu, vv, t20, rb21,
             p30, rb31, rb32, p40, p40b, rb41, rb42) = (
                T(t) for t in (
                    "r2 r ir rr lnr2 t1 t2 e2 e3 e4 uu vv t20 rb21 "
                    "p30 rb31 rb32 p40 p40b rb41 rb42"
                ).split()
            )
            vwu_t = pw.tile([128, 3, NCH], f32, tag="vwu", name="vwu")[:]
            ang5_t = pw.tile([128, 5, NCH], f32, tag="ang5", name="ang5")[:]

            def pointwise_half(h):
                """Compute psi into poly16[:, 16h:16h+16, :]."""
                sl = slice(16 * h, 16 * (h + 1))

                def H(t2d):
                    return t2d[:, sl]

                x, y, z = (xyz3[:, sl, t] for t in range(3))
                hr2, hr, hir, hrr, hlnr2 = (
                    H(t) for t in (r2, r, ir, rr, lnr2)
                )
                nc.vector.tensor_mul(hr2, x, x)
                tAh, tBh = H(uu), H(vv)   # reuse as scratch pre-uu/vv
                nc.vector.tensor_mul(tAh, y, y)
                nc.vector.tensor_add(hr2, hr2, tAh)
                nc.vector.tensor_mul(tBh, z, z)
                nc.vector.tensor_add(hr2, hr2, tBh)
                nc.scalar.activation(hlnr2, hr2, AF.Ln)
                nc.scalar.activation(hr, hlnr2, AF.Exp, scale=0.5)
                nc.scalar.activation(hir, hlnr2, AF.Exp, scale=-0.5)

                # radial exponentials from one exp: t1=e^{-r/12}
                ht1, ht2, he2, he3, he4 = (H(t) for t in (t1, t2, e2, e3, e4))
                nc.scalar.activation(ht1, hr, AF.Exp, scale=-1.0 / 12.0)
                nc.vector.tensor_mul(ht2, ht1, ht1)
                nc.vector.tensor_mul(he4, ht2, ht1)
                nc.vector.tensor_mul(he3, ht2, ht2)
                nc.vector.tensor_mul(he2, he3, ht2)

                vwu = vwu_t[:, :, sl]
                ang5 = ang5_t[:, :, sl]
                v, w, u = vwu[:, 0, :], vwu[:, 1, :], vwu[:, 2, :]
                uv, wv, a20, wu, a22 = (ang5[:, i, :] for i in range(5))
                nc.vector.tensor_mul(v, y, hir)
                nc.vector.tensor_mul(w, z, hir)
                nc.vector.tensor_mul(u, x, hir)
                nc.vector.tensor_mul(hrr, hr, hr)

                huu, hvv = H(uu), H(vv)
                nc.vector.tensor_mul(a20, w, w)
                nc.vector.tensor_scalar(a20, a20, 3.0, -1.0, ALU.mult, ALU.add)
                nc.vector.tensor_mul(huu, u, u)
                nc.vector.tensor_mul(hvv, v, v)
                nc.vector.tensor_sub(a22, huu, hvv)
                nc.vector.tensor_mul(uv, u, v)
                nc.vector.tensor_mul(wu, w, u)
                nc.vector.tensor_mul(wv, w, v)

                poly_s = poly16p[:, sl, 0:NB]
                slot = [poly16p[:, sl, k] for k in range(NB)]
                poly_kc = poly_s.rearrange("p c k -> p k c")

                nc.vector.tensor_mul(slot[0], he2, he2)          # e1
                ht20 = H(t20)
                nc.vector.tensor_scalar(ht20, hr, -1.0, 2.0, ALU.mult, ALU.add)
                nc.vector.tensor_mul(slot[1], ht20, he2)
                hrb21 = H(rb21)
                nc.vector.tensor_mul(hrb21, hr, he2)
                nc.vector.tensor_mul(poly_kc[:, 2:5, :], bcastn(hrb21, 3), vwu)
                hp30 = H(p30)
                nc.vector.tensor_scalar(hp30, hrr, 2.0 / 9.0, 3.0, ALU.mult, ALU.add)
                nc.vector.scalar_tensor_tensor(
                    hp30, hr, 2.0, hp30, ALU.mult, ALU.subtract
                )
                nc.vector.tensor_mul(slot[5], hp30, he3)
                hrb31 = H(rb31)
                nc.vector.tensor_scalar(
                    hrb31, hr, -4.0 / 9.0, 8.0 / 3.0, ALU.mult, ALU.add
                )
                nc.vector.tensor_mul(hrb31, hrb31, hr)
                nc.vector.tensor_mul(hrb31, hrb31, he3)
                nc.vector.tensor_mul(poly_kc[:, 6:9, :], bcastn(hrb31, 3), vwu)
                hrb32 = H(rb32)
                nc.vector.tensor_mul(hrb32, hrr, he3)
                nc.vector.tensor_mul(poly_kc[:, 9:14, :], bcastn(hrb32, 5), ang5)
                hp40, hp40b = H(p40), H(p40b)
                nc.vector.tensor_scalar(hp40, hr, -1.0 / 48.0, 0.5, ALU.mult, ALU.add)
                nc.vector.tensor_mul(hp40, hp40, hrr)
                nc.vector.tensor_scalar(hp40b, hr, -3.0, 4.0, ALU.mult, ALU.add)
                nc.vector.tensor_add(hp40, hp40, hp40b)
                nc.vector.tensor_mul(slot[14], hp40, he4)
                hrb41 = H(rb41)
                nc.vector.tensor_scalar(hrb41, hr, 1.0 / 16.0, -1.25, ALU.mult, ALU.add)
                nc.vector.tensor_mul(hrb41, hrb41, hr)
                nc.vector.tensor_scalar(hrb41, hrb41, 5.0, None, ALU.add)
                nc.vector.tensor_mul(hrb41, hrb41, hr)
                nc.vector.tensor_mul(hrb41, hrb41, he4)
                nc.vector.tensor_mul(poly_kc[:, 15:18, :], bcastn(hrb41, 3), vwu)
                hrb42 = H(rb42)
                nc.vector.tensor_scalar(hrb42, hr, -0.5, 6.0, ALU.mult, ALU.add)
                nc.vector.tensor_mul(hrb42, hrb42, hrr)
                nc.vector.tensor_mul(hrb42, hrb42, he4)
                nc.vector.tensor_mul(poly_kc[:, 18:23, :], bcastn(hrb42, 5), ang5)

            # ---- emission helpers ----------------------------------------
            def rep_half(h):
                sl = slice(16 * h, 16 * (h + 1))
                for rep in range(NBLK):
                    eng = nc.vector.tensor_copy if rep % 2 == 0 else nc.scalar.copy
                    eng(poly_rep[:, sl, rep, :], poly16p[:, sl, :])

            def transpose_half(h, drain_eng):
                tr = psum.tile([128, 2048], f32, tag="mm", name="tr_ps")
                trv = tr[:].bitcast(f16)
                for cc in range(16):
                    c = 16 * h + cc
                    nc.tensor.transpose(
                        trv[:, 128 * cc:128 * (cc + 1)],
                        poly_rep[:, c, :, :],
                        ident[:],
                    )
                drain_eng(poly_t4[:, ts(h, 2048)], trv[:, 0:2048])

            # per (mt, h): 4 strip-concurrent MMs; strip i computes point
            # window w=4h+i (psum bank i == stage cols 2048h+512i == points
            # 2048h+512i.. -- identity order), one copy, one 512 KiB DMA.
            def mm_group(mt, h, use_scalar):
                ps = psum.tile([128, 2048], f32, tag="mm", name="mmps")
                for i in range(NBLK):
                    w = 4 * h + i
                    nc.tensor.matmul(
                        ps[:, ts(i, 512)],
                        lhsT=coefft4[32 * i:32 * i + NB, ts(mt, 128)],
                        rhs=poly_t4[32 * i:32 * i + NB, ts(w, 512)],
                        start=True,
                        stop=True,
                        tile_position=(32 * i, 0),
                    )
                stage = stage_pool.tile([128, 2048], f16, tag="stage", name="stage")
                if use_scalar:
                    nc.scalar.copy(stage[:], ps[:])
                else:
                    nc.vector.tensor_copy(stage[:], ps[:])
                nc.sync.dma_start(
                    out=out_d[ts(mt, 128), ts(h, 2048)], in_=stage[:]
                )

            # ---- emission order: h0 chain; 4 early h0 groups run on the
            # PE between the two transpose batches; h1 groups lag h0 by 4.
            pointwise_half(0)
            rep_half(0)
            transpose_half(0, nc.scalar.copy)
            pointwise_half(1)
            rep_half(1)
            for mt in range(4):
                mm_group(mt, 0, use_scalar=(mt % 2 == 0))
            transpose_half(1, nc.vector.tensor_copy)

            order = [(m, 1) for m in range(4)]
            for m in range(4, NMT):
                order.append((m, 0))
                order.append((m, 1))
            scal_cum = 0
            n_rest = len(order)
            for idx, (mt, h) in enumerate(order):
                s_next = ((idx + 1) * (SCALAR_COPIES - 2)) // n_rest
                use_scalar = s_next > scal_cum
                scal_cum = s_next
                mm_group(mt, h, use_scalar)

    nc.finalize()
    return nc


def _get_program():
    global _PROGRAM
    if _PROGRAM is None:
        _PROGRAM = _build_program()
    return _PROGRAM


def _prep_inputs(position, coefficients):
    pos = np.asarray(position, dtype=np.float32)
    coeff = np.asarray(coefficients, dtype=np.float32)
    assert pos.shape == (B, PTS, 3) and coeff.shape == (OUTC, INC, NB)
    c = _basis_scales().astype(np.float32)
    C = (coeff * c).reshape(MN, NB).T.astype(np.float16)  # [23, 4096]
    coefft4 = np.zeros((128, MN), dtype=np.float16)
    for i in range(NBLK):
        coefft4[32 * i:32 * i + NB] = C
    # point q -> (partition q%128, chunk q//128)
    return [
        {
            "position": np.ascontiguousarray(
                pos[b].reshape(NCH, 128, 3).transpose(1, 0, 2).reshape(128, 96)
            ),
            "coefft4": coefft4,
        }
        for b in range(B)
    ]


def _assemble(results):
    return np.stack(
        [
            np.asarray(r["out"]).astype(np.float32).reshape(OUTC, INC, PTS)
            for r in results
        ],
        axis=2,
    )


def kernel(position, coefficients):
    from concourse import bass_utils

    nc = _get_program()
    in_maps = _prep_inputs(position, coefficients)
    res = bass_utils.run_bass_kernel_spmd(nc, in_maps, core_ids=list(range(NCORES)))
    return _assemble(res.results)


def kernel_traced(position, coefficients, trace_cores=None):
    """Like kernel() but captures an NTFF trace; returns (out, results)."""
    from concourse import bass_utils

    nc = _get_program()
    in_maps = _prep_inputs(position, coefficients)
    res = bass_utils.run_bass_kernel_spmd(
        nc,
        in_maps,
        core_ids=list(range(NCORES)),
        trace=True,
        trace_cores=trace_cores,
    )
    return _assemble(res.results), res
